# revision 16
# baseline (speedup 1.0000x reference)
"""Multi-head attention (B=2, N=2048, D=1024, H=16) on 8 TRN2 NeuronCores.

Sharding: tensor-parallel over heads. Core i handles global heads
[2i, 2i+2) for both batches; after attention, one 8-core AllToAll
re-shards from head-split to (batch, sequence-slab)-split and core j
computes the output projection for batch j//4, rows [512*(j%4), ...).

Device-side design (all matmuls f16 with f32 PSUM accumulation):
  - Q^T/K^T [d, n] from weight-stationary matmuls against x^T. The 1/8
    softmax scale and bq are folded into wq/bq on host; bk is DROPPED
    (a per-query constant in the scores, softmax-invariant); bv is
    folded into bo on host (bo' = bv @ wo + bo).
  - Scores S^T [k, q] per 128-key chunk: the two local heads run
    CONCURRENTLY in 64x128 row-tiled PE mode (head h's K/Q live on
    SBUF partitions [64h, 64h+64) -> tile_position (64h, 0)).
  - exp on the Scalar engine in 9 instruction groups per slab
    (sizes 4,3,4,3,4,3,4,3,4 chunks, alternating heads) -> big-N
    activations; PSUM: 4-bank + 3-bank rotating slots + 1 AV bank.
  - A*V: per key-chunk the two heads run CONCURRENTLY in 128x64
    column-tiled mode into one PSUM bank (h0 -> partitions 0:64,
    h1 -> 64:128), accumulating over all 16 chunks.
  - Softmax denominators: DVE accumulates per-head partial sums of
    exp chunks ([128, 512] f16), then a gpsimd cross-partition
    tensor_reduce gives [1, 512]; reciprocal + broadcast + multiply
    normalize the AV output. No extra matmuls or PSUM banks.
  - QKV of batch 1 is software-pipelined into batch 0's attention
    (the Scalar engine is the bottleneck; PE has slack).
  - One merged AllToAll [8, 128, 512] f16 re-shards; output projection
    in 128x128 mode + bias, DMA out.
"""

import ml_dtypes
import numpy as np

import concourse.bass as bass
import concourse.bass_isa as bass_isa
import concourse.mybir as mybir
import concourse.tile as tile
from concourse import bacc
from concourse.bass_utils import run_bass_kernel_spmd
from concourse.masks import make_identity

F32 = mybir.dt.float32
F16 = mybir.dt.float16
EXP = mybir.ActivationFunctionType.Exp
BYPASS = mybir.AluOpType.bypass
ADD = mybir.AluOpType.add
AXC = mybir.AxisListType.C

P = 128
B, N, D = 2, 2048, 1024
NH, HD = 16, 64
TP = 8                  # head-parallel group size (all cores)
HL = D // TP            # 128 local head dims (2 heads x 64)
NHL = NH // TP          # 2 local heads
QS = 512                # query slab width
NQS = N // QS           # 4 slabs per batch
KC = N // P             # 16 key chunks of 128
DC = D // P             # 8 model-dim chunks of 128
ROWS = 512              # output rows per core (one slab of one batch)
RG = [[0, 1, 2, 3, 4, 5, 6, 7]]
SCALE = 1.0 / np.sqrt(HD)

# slabs in processing order; slab index s -> (batch, q-slab) = dest core s
SLABS = [(0, 0), (0, 1), (0, 2), (0, 3), (1, 0), (1, 1), (1, 2), (1, 3)]
# Scores are emitted as PAIRS of exp-groups, one per head, with the two
# heads' matmuls INTERLEAVED in issue order so the 64x128 row tiles run
# concurrently. h0 groups use the 4-bank PSUM slot, h1 the 3-bank slot
# (except the last pair, where h1's 4-chunk tail takes the 4-bank slot).
SCHED = [((0, 4), (0, 3)), ((4, 8), (3, 6)), ((8, 12), (6, 9)),
         ((12, 16), (9, 12)), (None, (12, 16))]

_CACHE = {}


def build_nc(reps=1):
    nc = bacc.Bacc("TRN2", target_bir_lowering=False, debug=False,
                   num_devices=8)

    xt_ext = nc.declare_dram_parameter("xt", [B, D, N], F16, isOutput=False)
    wq_ext = nc.declare_dram_parameter("wq", [P, DC, HL], F16, isOutput=False)
    bq_ext = nc.declare_dram_parameter("bq", [P, 1], F32, isOutput=False)
    wk_ext = nc.declare_dram_parameter("wk", [P, DC, HL], F16, isOutput=False)
    wv_ext = nc.declare_dram_parameter("wv", [P, DC, HL], F16, isOutput=False)
    wo_ext = nc.declare_dram_parameter("wo", [P, DC, D], F16, isOutput=False)
    bo_ext = nc.declare_dram_parameter("bo", [1, D], F32, isOutput=False)
    out_ext = nc.declare_dram_parameter("out", [ROWS, D], F32, isOutput=True)

    with tile.TileContext(nc) as tc:
        with (
            tc.tile_pool(name="const", bufs=1) as const,
            tc.tile_pool(name="persist", bufs=1) as persist,
            tc.tile_pool(name="dram", bufs=1, space="DRAM") as dram,
            tc.tile_pool(name="xtp", bufs=1) as xtp,
            tc.tile_pool(name="wp", bufs=1) as wp,
            tc.tile_pool(name="wo_p", bufs=1) as wo_p,
            tc.tile_pool(name="vtp", bufs=2) as vtp,
            tc.tile_pool(name="ptp", bufs=2) as ptp,
            tc.tile_pool(name="prt", bufs=2) as prt,
            tc.tile_pool(name="nrm", bufs=4) as nrm,
            tc.tile_pool(name="psS4", bufs=1, space="PSUM") as psS4,
            tc.tile_pool(name="psS3", bufs=1, space="PSUM") as psS3,
            tc.tile_pool(name="psAV", bufs=1, space="PSUM") as psAV,
        ):
            identity_h = const.tile([P, P], F16)
            make_identity(nc, identity_h)
            zeros_h = const.tile([P, P], F16)
            nc.gpsimd.memset(zeros_h[:], 0.0)

            # persistent SBUF tensors: d-on-partitions, head h at [64h, 64h+64)
            QT = persist.tile([P, B, N], F16)
            KT = persist.tile([P, B, N], F16)
            Vr = persist.tile([P, B, KC, NHL, HD], F16)   # [k, b, kc, h, d]

            a2a_in = dram.tile([TP, P, QS], F16, name="a2a_in")
            a2a_out = dram.tile([TP, P, QS], F16, name="a2a_out")

            wq_sb = wp.tile([P, DC, HL], F16)
            wk_sb = wp.tile([P, DC, HL], F16)
            wv_sb = wp.tile([P, DC, HL], F16)
            for w_sb, w_ext in ((wq_sb, wq_ext), (wk_sb, wk_ext),
                                (wv_sb, wv_ext)):
                nc.sync.dma_start(w_sb, w_ext[:])
            bq_sb = wp.tile([P, 1], F32)   # pre-scaled by 1/sqrt(HD) on host
            nc.sync.dma_start(bq_sb, bq_ext[:])

            # PSUM slots: one 4-bank + one 3-bank (+1 AV bank = 8 total).
            # >3-bank tiles (the 4-chunk score groups) must use the 4-bank
            # slot; smaller tiles rotate so consecutive users ping-pong.
            slot_state = {"i": 0}

            def slot_tile(shape, dtype):
                nbytes = int(np.prod(shape[1:])) * mybir.dt.size(dtype)
                if nbytes > 3 * QS * 4:
                    pool, tag = psS4, "s4"
                    slot_state["i"] = 1   # next small tile lands on S3
                else:
                    slot_state["i"] ^= 1
                    pool, tag = ((psS3, "s3"), (psS4, "s4"))[slot_state["i"]]
                assert nbytes <= (4 if tag == "s4" else 3) * QS * 4
                return pool.tile(shape, dtype, tag=tag, name=f"ps_{tag}")

            def load_xt(b, xT):
                # chunked so the first matmuls start when D-chunk 0 lands
                for dc in range(DC):
                    for half in range(2):
                        nc.sync.dma_start(
                            xT[:, dc, half * (N // 2):(half + 1) * (N // 2)],
                            xt_ext[b, dc * P:(dc + 1) * P,
                                   half * (N // 2):(half + 1) * (N // 2)])

            def proj(kind, b, qs, xT):
                w_sb = {"q": wq_sb, "k": wk_sb}[kind]
                psm = slot_tile([P, QS], F32)
                for dc in range(DC):
                    nc.tensor.matmul(
                        psm, lhsT=w_sb[:, dc, :],
                        rhs=xT[:, dc, qs * QS:(qs + 1) * QS],
                        start=(dc == 0), stop=(dc == DC - 1))
                dst = (QT if kind == "q" else KT)[:, b, qs * QS:(qs + 1) * QS]
                if kind == "q":
                    nc.vector.tensor_scalar(dst, psm, bq_sb[:, 0:1], None, ADD)
                else:
                    nc.vector.tensor_copy(dst, psm)

            def vproj(b, ks, xT):
                # V^T [d, 512 keys] then PE-transpose per 128-chunk into Vr
                psm = slot_tile([P, QS], F32)
                for dc in range(DC):
                    nc.tensor.matmul(
                        psm, lhsT=wv_sb[:, dc, :],
                        rhs=xT[:, dc, ks * QS:(ks + 1) * QS],
                        start=(dc == 0), stop=(dc == DC - 1))
                vt = vtp.tile([P, QS], F16, tag="vt", name="vt")
                nc.vector.tensor_copy(vt, psm)
                pst = slot_tile([P, 4, P], F16)
                for kk in range(QS // P):
                    nc.tensor.transpose(
                        pst[:, kk, :], vt[:, kk * P:(kk + 1) * P], identity_h)
                nc.vector.tensor_copy(
                    out=Vr[:, b, ks * 4:(ks + 1) * 4, :, :],
                    in_=pst[:].rearrange("p c (h d) -> p c h d", h=NHL, d=HD))

            def score_mm(b, qs, h, kc, dst):
                nc.tensor.matmul(
                    dst,
                    lhsT=KT[h * HD:(h + 1) * HD, b, kc * P:(kc + 1) * P],
                    rhs=QT[h * HD:(h + 1) * HD, b, qs * QS:(qs + 1) * QS],
                    start=True, stop=True)

            def partial_adds(h, c0, c1, pt, partials):
                pa = partials[h]
                for kc in range(c0, c1):
                    if c0 == 0 and kc == 0:
                        continue
                    if c0 == 0 and kc == 1:
                        nc.vector.tensor_add(pa, pt[h][:, 0, :], pt[h][:, 1, :])
                    else:
                        nc.vector.tensor_add(pa, pa, pt[h][:, kc, :])

            def scores_pair(s, pi, pt, partials):
                b, qs = SLABS[s]
                g0, g1 = SCHED[pi]
                d0, d1 = g1
                if g0 is not None:
                    c0, c1 = g0
                    t4 = psS4.tile([P, c1 - c0, QS], F32, tag="s4", name="s4")
                    t3 = psS3.tile([P, d1 - d0, QS], F32, tag="s3", name="s3")
                else:
                    c0, c1, t4 = 0, 0, None
                    t3 = psS4.tile([P, d1 - d0, QS], F32, tag="s4", name="s4h")
                # interleave the two heads' matmuls so the row tiles overlap
                for i in range(max(c1 - c0, d1 - d0)):
                    if c0 + i < c1:
                        score_mm(b, qs, 0, c0 + i, t4[:, i, :])
                    if d0 + i < d1:
                        score_mm(b, qs, 1, d0 + i, t3[:, i, :])
                if t4 is not None:
                    nc.scalar.activation(pt[0][:, c0:c1, :], t4, EXP)
                nc.scalar.activation(pt[1][:, d0:d1, :], t3, EXP)
                if t4 is not None:
                    partial_adds(0, c0, c1, pt, partials)
                partial_adds(1, d0, d1, pt, partials)

            def attn_av(s, pt, partials):
                b, qs = SLABS[s]
                av = psAV.tile([P, QS], F32, tag="av", name="av")
                # The two heads accumulate CONCURRENTLY (column tiles) into
                # disjoint partition halves of one bank. start=True would
                # clear has_written for the whole bank (wiping the other
                # chain), so clear once with a tiny N=1 zero matmul in the
                # same 128x64 col-tiled mode, then accumulate with
                # start=False throughout (the first write per element lands
                # as overwrite since its bit is clear).
                for h in range(NHL):
                    nc.tensor.matmul(av[h * HD:(h + 1) * HD, 0:1],
                                     lhsT=zeros_h[:, 0:HD],
                                     rhs=zeros_h[:, 0:1], start=True,
                                     stop=False, skip_group_check=True)
                for kc in range(KC):
                    for h in range(NHL):
                        nc.tensor.matmul(
                            av[h * HD:(h + 1) * HD, :],
                            lhsT=Vr[:, b, kc, h, :],
                            rhs=pt[h][:, kc, :],
                            start=False, stop=(kc == KC - 1),
                            skip_group_check=True)
                onrm = nrm.tile([P, QS], F16, tag="onrm", name="onrm")
                for h in range(NHL):
                    den = nrm.tile([P, QS], F32, tag="den", name="den")
                    nc.gpsimd.partition_all_reduce(
                        den, partials[h], channels=P,
                        reduce_op=bass_isa.ReduceOp.add)
                    rec = nrm.tile([1, QS], F16, tag="rec", name="rec")
                    with nc.allow_low_precision(
                            reason="softmax denom reciprocal to f16"):
                        nc.vector.reciprocal(rec, den[0:1, :])
                    bc = nrm.tile([HD, QS], F16, tag="bc", name="bc")
                    nc.gpsimd.partition_broadcast(bc[:], rec[:])
                    nc.vector.tensor_mul(
                        onrm[h * HD:(h + 1) * HD, :],
                        av[h * HD:(h + 1) * HD, :], bc)
                nc.sync.dma_start(a2a_in[s], onrm)

            for _rep in range(reps):
                xT0 = xtp.tile([P, DC, N], F16, tag="xT", name="xT0")
                load_xt(0, xT0)

                pt_s = {}       # live pt / partial tiles per in-flight slab
                pr_s = {}

                def new_slab(s):
                    pt_s[s] = [ptp.tile([P, KC, QS], F16, tag=f"pt{h}",
                                        name=f"pt{h}")
                               for h in range(NHL)]
                    pr_s[s] = [prt.tile([P, QS], F16, tag=f"pa{h}",
                                        name=f"pa{h}")
                               for h in range(NHL)]

                def pairs(s, lo, hi):
                    for pi in range(lo, hi):
                        scores_pair(s, pi, pt_s[s], pr_s[s])

                def finish(s):
                    attn_av(s, pt_s.pop(s), pr_s.pop(s))

                # ---- lead-in: qkv(b0) interleaved with slab 0 scores ----
                new_slab(0)
                proj("q", 0, 0, xT0)
                proj("k", 0, 0, xT0)
                pairs(0, 0, 1)
                proj("k", 0, 1, xT0)
                pairs(0, 1, 2)
                proj("k", 0, 2, xT0)
                pairs(0, 2, 3)
                proj("k", 0, 3, xT0)
                pairs(0, 3, 5)

                proj("q", 0, 1, xT0)
                vproj(0, 0, xT0)
                vproj(0, 1, xT0)
                new_slab(1)
                pairs(1, 0, 1)
                vproj(0, 2, xT0)
                vproj(0, 3, xT0)
                pairs(1, 1, 2)
                finish(0)
                pairs(1, 2, 5)
                proj("q", 0, 2, xT0)

                new_slab(2)
                pairs(2, 0, 2)
                finish(1)
                pairs(2, 2, 5)
                proj("q", 0, 3, xT0)
                xT1 = xtp.tile([P, DC, N], F16, tag="xT", name="xT1")
                load_xt(1, xT1)
                proj("k", 1, 0, xT1)
                wo_sb = wo_p.tile([P, DC, D], F16, tag="wo_sb", name="wo_sb")
                nc.sync.dma_start(wo_sb, wo_ext[:])
                bo_sb = wo_p.tile([1, D], F32, tag="bo_sb", name="bo_sb")
                nc.sync.dma_start(bo_sb, bo_ext[:])
                bo_bc = wo_p.tile([P, D], F32, tag="bo_bc", name="bo_bc")
                nc.gpsimd.partition_broadcast(bo_bc[:], bo_sb[:])

                new_slab(3)
                pairs(3, 0, 2)
                finish(2)
                pairs(3, 2, 5)
                proj("k", 1, 1, xT1)
                proj("k", 1, 2, xT1)
                proj("k", 1, 3, xT1)
                proj("q", 1, 0, xT1)

                new_slab(4)
                pairs(4, 0, 2)
                finish(3)
                pairs(4, 2, 5)
                vproj(1, 0, xT1)
                vproj(1, 1, xT1)
                vproj(1, 2, xT1)
                vproj(1, 3, xT1)
                proj("q", 1, 1, xT1)

                new_slab(5)
                pairs(5, 0, 2)
                finish(4)
                pairs(5, 2, 5)
                proj("q", 1, 2, xT1)

                new_slab(6)
                pairs(6, 0, 2)
                finish(5)
                pairs(6, 2, 5)
                proj("q", 1, 3, xT1)

                new_slab(7)
                pairs(7, 0, 2)
                finish(6)
                pairs(7, 2, 5)
                finish(7)

                nc.gpsimd.collective_compute(
                    "AllToAll", BYPASS,
                    ins=[a2a_in[:].opt()],
                    outs=[a2a_out[:].opt()],
                    replica_groups=RG)

                # ---------------- output projection ----------------
                ot_sb = wo_p.tile([P, DC, QS], F16, tag="ot_sb", name="ot_sb")
                for src in range(TP):
                    nc.sync.dma_start(ot_sb[:, src, :], a2a_out[src])
                for mq in range(ROWS // P):
                    for oc in range(2):
                        psm = slot_tile([P, QS], F32)
                        for dc in range(DC):
                            nc.tensor.matmul(
                                psm,
                                lhsT=ot_sb[:, dc, mq * P:(mq + 1) * P],
                                rhs=wo_sb[:, dc, oc * QS:(oc + 1) * QS],
                                start=(dc == 0), stop=(dc == DC - 1))
                        o_t = nrm.tile([P, QS], F32, tag="ot", name="o_t")
                        nc.vector.tensor_add(
                            out=o_t, in0=psm,
                            in1=bo_bc[:, oc * QS:(oc + 1) * QS])
                        nc.sync.dma_start(
                            out_ext[mq * P:(mq + 1) * P,
                                    oc * QS:(oc + 1) * QS], o_t)

    nc.finalize()
    return nc


def _chunked(w):
    # [D, n] -> [P, DC, n]: row r = c*P + p lands at [p, c]
    n = w.shape[1]
    return np.ascontiguousarray(w.reshape(DC, P, n).transpose(1, 0, 2))


def make_in_maps(inputs):
    f16 = np.float16
    x = np.asarray(inputs["x"], dtype=np.float32)
    xt = np.ascontiguousarray(x.transpose(0, 2, 1)).astype(f16)
    wq = np.asarray(inputs["wq"], np.float32)
    wk = np.asarray(inputs["wk"], np.float32)
    wv = np.asarray(inputs["wv"], np.float32)
    wo = np.asarray(inputs["wo"], np.float32)
    bq = np.asarray(inputs["bq"], np.float32)
    bv = np.asarray(inputs["bv"], np.float32)
    bo = np.asarray(inputs["bo"], np.float32)
    # fold the softmax scale into wq/bq; bk is softmax-invariant (dropped);
    # fold bv through the output projection into bo
    wq_s = (wq * SCALE).astype(f16)
    bq_s = (bq * SCALE).astype(np.float32)
    bo2 = (bv @ wo + bo).reshape(1, D).astype(np.float32)
    wo_r = _chunked(wo.astype(f16))
    in_maps = []
    for i in range(8):
        hs = i * HL
        m = {"xt": xt,
             "wq": _chunked(wq_s[:, hs:hs + HL]),
             "wk": _chunked(wk.astype(f16)[:, hs:hs + HL]),
             "wv": _chunked(wv.astype(f16)[:, hs:hs + HL]),
             "bq": np.ascontiguousarray(bq_s[hs:hs + HL].reshape(1, P).T),
             "wo": wo_r,
             "bo": bo2}
        in_maps.append(m)
    return in_maps


def kernel(**inputs):
    if "nc" not in _CACHE:
        _CACHE["nc"] = build_nc()
    nc = _CACHE["nc"]
    in_maps = make_in_maps(inputs)
    res = run_bass_kernel_spmd(nc, in_maps, core_ids=list(range(8)))
    out = np.empty((B, N, D), dtype=np.float32)
    for j in range(8):
        b, t = j // NQS, j % NQS
        out[b, t * ROWS:(t + 1) * ROWS] = res.results[j]["out"]
    return out
